# revision 79
# baseline (speedup 1.0000x reference)
"""Trainium2 Bass kernel for DirectionalSeparableConv2D (v4).

Full-input contract: kernel(**inputs) takes the complete unsharded inputs
(x [128,128,48,48] plus the small weight tensors) and returns the full
[128,128,48,48] output. Internally shards batch 16-per-core across 8
NeuronCores (SPMD), with all weights replicated.

Design (vs the ~168-178us v3 baseline; measures ~116-120us):
  - Padded DRAM/SBUF x layout: rows are W+2=50 wide and image/group
    segments are separated by 100-col zero gaps, so every shifted tap
    (PE matmul rhs or vector op src) reads zeros at the SAME-conv
    borders.  No border clipping anywhere: every PSUM write is a full
    contiguous 384-col block and every elementwise tap is one flat
    dense 1D op (eligible for the DVE 2x/4x packed modes).
  - Engine split (LP-balanced): PE runs 20 packs per 8-row chunk
    (9 central taps, 7 fused dir taps, 4 stage-2 mixes) as 4-way
    tile_position-concurrent matmuls (~3.9 cols/cycle measured); DVE
    runs the remaining dir taps as tensor_scalar 4x muls + bf16
    tensor_tensor 2x adds + two 1x STT MACs; ACT does the PSUM
    evacuation and two tap muls.  GpSimd is left idle ON PURPOSE: its
    SBUF port is shared with DVE and concurrent GpSimd tensor ops
    halve DVE throughput (measured ~2.3x slowdown on overlapped ops).
  - Center-tap folding: y''_g = x + sum_t (k_t/k2) shift_t(x); the
    stage-2 weight columns are pre-scaled by k2[c] on the host, so the
    center taps cost nothing on-device.
  - DMA: whole-block loads ([128, 5.2KB] cen / [128, 20.2KB] dir
    lines, all 16 SDMA engines engaged) and device-native stores
    ([128, 6KB] lines) that the host unpermutes; ~25 GB/s per
    descriptor stream vs ~14 with the v3 layout (2.2KB packets).
  - Pipeline fill: block 0's chunks 0-2 run fully PE-fused (all 20 dir
    taps as matmuls, no stage-2) so nothing gates on the serial DVE
    y''-prep chain while the pipe fills; its y''-prep covers only rows
    24-48.  ACT FIFO is strict, so cross-block ACT ops that wait on a
    future DMA must not precede this block's evacuations; yprep for
    block 1 runs all-DVE for that reason.
"""

import numpy as np

import concourse.bacc as bacc
import concourse.mybir as mybir
import concourse.tile as tile
from concourse.bass_utils import run_bass_kernel_spmd

F32 = mybir.dt.float32
BF16 = mybir.dt.bfloat16

# Problem shapes (hardcoded per contract).
B, C, H, W = 128, 128, 48, 48
HW = H * W
CEN_IN, DIR_IN = 32, 24
N_CORES = 8

# Per-core tiling.
NB = B // N_CORES          # images per core (16)
IPB = 4                    # images per block (one per 32-partition slot)
NBLK = NB // IPB           # 4 blocks
RPC = 8                    # rows per PSUM chunk
CHUNK = RPC * W            # 384
PCH = 512                  # PSUM per-image slice stride (bank aligned)
NCH = H // RPC             # 6 chunks per image
SGRP = 2                   # chunks buffered per output store
PR = 96 + DIR_IN           # partition extent covering all dir slots (120)

# Padded plane geometry: rows are PW=50 wide (2 pad cols absorb dx in
# [-2,2] reads); segments start with 100 zero cols (absorb dy in [-2,2]
# row reads); DOFF is the offset of pixel (0,0) within a segment.
PW = W + 2                 # 50
GS = 100 + 2 + H * PW      # 2502 segment stride
DOFF = 102                 # data base within a segment
CENF = GS + 100            # cen tile free size (1 segment + tail zeros)
DIRF = 4 * GS + 104        # dir tile free size (4 group segments + tail)
NROW = H * PW              # 2400 flat cols per plane (incl in-row pads)

# Dir-group tap geometry: group g shift for tap t (t=0..4, center t=2).
#   g=0 horizontal (0, t-2); g=1 vertical (t-2, 0);
#   g=2 diagonal (t-2, t-2); g=3 anti-diagonal (t-2, 2-t).
def dir_shift(g, t):
    d = t - 2
    return [(0, d), (d, 0), (d, d), (d, -d)][g]

# PE keeps 8 dir taps fused with the mix (LP-balanced against DVE/ACT);
# the remaining 8 taps (plus the 4 folded centers) run elementwise:
#   fold-dve:   y = k~t*shift(x)  (tensor_scalar mul 4x; even offsets)
#   fold-act:   y = k~t*shift(x)  (ACT mul; handles odd offsets)
#   then        y += x            (DVE tensor_tensor add, 2x)
#   mac-dve:    tmp = k~t*shift(x) (4x); y += tmp (2x)
#   mac-stt:    y = k~t*shift(x) + y  (DVE STT 1x; odd offsets)
# GpSimd stays idle: its SBUF port is shared with DVE and concurrent
# GpSimd tensor ops halve DVE throughput (measured).
PE_TAPS = [(0, 0), (0, 1), (0, 3), (1, 0), (1, 1), (1, 3), (2, 0)]
# folds: (g, t, engine); first elementwise tap per group, absorbs y=..
EW_FOLDS = [(1, 4, 'dve'), (3, 0, 'dve'), (0, 4, 'act'), (2, 3, 'act')]
# macs: (g, t, form); tmul = DVE ts-mul + add, smul = ACT mul + DVE add
EW_MACS = [(3, 4, 'tmul'), (3, 1, 'stt'), (3, 3, 'stt'),
           (2, 4, 'tmul'), (2, 1, 'stt')]
SCAL_TAPS = [0, 1, 3, 4]   # scalar columns (k_t/k2) used by elementwise taps

# bf16 weight bundle layout (columns).  The fused-dir section holds ALL
# 20 (g, t) taps: blocks 1-3 index the PE_TAPS subset; block 0 runs
# fully fused on the PE (no y''/stage-2) to avoid gating its chunks on
# the serial DVE y''-prep chain during pipeline fill.
WB_CEN = 0                 # 9 central taps x 128 cols
WB_DIR = 9 * 128           # 20 fused dir taps x 128 cols
WB_S2 = WB_DIR + 20 * 128  # 4 stage-2 blocks x 128 cols
NWTB = WB_S2 + 4 * 128


def build_mix(cen2cen, par2cen, dia2cen, cen2dir, dir2dir):
    mix = np.zeros((C, C), np.float32)
    mix[0:32, 0:32] = cen2cen
    mix[0:32, 32:56] = par2cen
    mix[0:32, 56:80] = par2cen
    mix[0:32, 80:104] = dia2cen
    mix[0:32, 104:128] = dia2cen
    for g in range(4):
        r = 32 + 24 * g
        mix[r:r + 24, 0:32] = cen2dir
        mix[r:r + 24, r:r + 24] = dir2dir
    return mix


def build_weights(cen_tensor, dir_tensor, cen2cen, par2cen, dia2cen, cen2dir, dir2dir):
    mix = build_mix(cen2cen, par2cen, dia2cen, cen2dir, dir2dir)
    bf = mybir.dt.np(BF16)
    # Clamp the center tap away from 0 so the fold ratios stay finite;
    # using the clamped value consistently in both the ratios and the
    # stage-2 scale keeps the math exact up to the 1e-8 perturbation.
    k2 = dir_tensor[:, 2]
    k2c = np.where(np.abs(k2) < 1e-8, np.where(k2 < 0, -1e-8, 1e-8), k2)
    wtb = np.zeros((128, NWTB), bf)
    for t in range(9):
        blk = (mix[:, 0:32] * cen_tensor[:, t // 3, t % 3][None, :]).T.astype(bf)
        for i in range(IPB):
            wtb[32 * i:32 * i + 32, WB_CEN + 128 * t:WB_CEN + 128 * (t + 1)] = blk
    for g in range(4):
        for t in range(5):
            j = 5 * g + t
            cols = slice(32 + 24 * g, 56 + 24 * g)
            blk = (mix[:, cols] * dir_tensor[:, t][None, :]).T.astype(bf)
            for i in range(IPB):
                wtb[32 * i:32 * i + 24,
                    WB_DIR + 128 * j:WB_DIR + 128 * (j + 1)] = blk
    for g in range(4):
        s2 = (mix[:, 32 + 24 * g:56 + 24 * g] * k2c[None, :]).T.astype(bf)
        for i in range(IPB):
            wtb[32 * i:32 * i + 24, WB_S2 + 128 * g:WB_S2 + 128 * (g + 1)] = s2
    # f32 per-partition scalar columns: k_t/k2 for the elementwise taps.
    wts = np.zeros((128, len(SCAL_TAPS)), np.float32)
    for j, t in enumerate(SCAL_TAPS):
        for i in range(IPB):
            wts[32 * i:32 * i + 24, j] = dir_tensor[:, t] / k2c
    return wtb, wts


def build_nc(nb=NB):
    """Emit the per-core Bass program for nb images."""
    assert nb % IPB == 0
    nblk = nb // IPB
    nc = bacc.Bacc("TRN2", target_bir_lowering=False, debug=False)

    xcen = nc.dram_tensor("xcen", [nblk, 128, CENF], BF16, kind="ExternalInput")
    xdir = nc.dram_tensor("xdir", [nblk, 128, DIRF], BF16, kind="ExternalInput")
    wtbd = nc.dram_tensor("wtb", [128, NWTB], BF16, kind="ExternalInput")
    wtsd = nc.dram_tensor("wts", [128, len(SCAL_TAPS)], F32, kind="ExternalInput")
    # Device-native output: staging tiles dumped verbatim; host unpermutes.
    out = nc.dram_tensor("out", [128, nblk * (NCH // SGRP) * IPB * SGRP * CHUNK],
                         BF16, kind="ExternalOutput")

    MULT = mybir.AluOpType.mult
    ADD = mybir.AluOpType.add

    with tile.TileContext(nc) as tc:
        with (
            tc.tile_pool(name="wpool", bufs=1) as wpool,
            tc.tile_pool(name="cpool", bufs=4) as cpool,
            tc.tile_pool(name="dpool", bufs=4) as dpool,
            tc.tile_pool(name="ypool", bufs=2) as ypool,
            tc.tile_pool(name="tpool", bufs=2) as tpool,
            tc.tile_pool(name="spool", bufs=2) as spool,
            tc.tile_pool(name="ppool", bufs=2, space="PSUM") as ppool,
        ):
            wtb = wpool.tile([128, NWTB], BF16)
            # central-tap weight columns first: they gate the first matmuls
            nc.scalar.dma_start(out=wtb[:, 0:WB_DIR], in_=wtbd[:, 0:WB_DIR])
            nc.scalar.dma_start(out=wtb[:, WB_DIR:NWTB], in_=wtbd[:, WB_DIR:NWTB])
            wts = wpool.tile([128, len(SCAL_TAPS)], F32)
            nc.scalar.dma_start(out=wts[:, :], in_=wtsd[:, :])
            scal = {t: wts[0:PR, j:j + 1] for j, t in enumerate(SCAL_TAPS)}

            cen4_t, dir4_t, y4_t = {}, {}, {}

            def emit_loads(b):
                cen4 = cpool.tile([128, CENF], BF16, name=f"cen4_{b}", tag="cen4")
                dir4 = dpool.tile([128, DIRF], BF16, name=f"dir4_{b}", tag="dir4")
                cen4_t[b], dir4_t[b] = cen4, dir4
                if b == 0:
                    # chunk 0 reads cen rows <= 9 only: land those first
                    cs = DOFF + 10 * PW
                    nc.sync.dma_start(out=cen4[:, 0:cs], in_=xcen[b, :, 0:cs])
                    nc.sync.dma_start(out=cen4[:, cs:CENF],
                                      in_=xcen[b, :, cs:CENF])
                else:
                    nc.sync.dma_start(out=cen4[:, :], in_=xcen[b, :, :])
                if b == 0:
                    for g in range(4):
                        hi = (g + 1) * GS if g < 3 else DIRF
                        nc.sync.dma_start(out=dir4[:, g * GS:hi],
                                          in_=xdir[b, :, g * GS:hi])
                else:
                    nc.sync.dma_start(out=dir4[:, :], in_=xdir[b, :, :])

            def make_yprep(b, slices=((0, 800), (800, 2400)), use_act=False,
                           all_dve=False):
                """Build the y''-prep op list (emission deferred)."""
                dir4 = dir4_t[b]
                y4 = ypool.tile([128, DIRF], BF16, name=f"y4_{b}", tag="y4")
                y4_t[b] = y4
                tmp = tpool.tile([128, NROW], BF16, name=f"tmp_{b}", tag="tmp")
                ops = []

                def aps(g, t, f0, n):
                    dy, dx = dir_shift(g, t)
                    base = g * GS + DOFF + f0
                    off = dy * PW + dx
                    return (dir4[0:PR, base + off:base + off + n],
                            dir4[0:PR, base:base + n],
                            y4[0:PR, base:base + n])

                for f0, f1 in slices:
                    n = f1 - f0
                    for g, t, eng in EW_FOLDS:
                        src, _, dst = aps(g, t, f0, n)
                        if eng == 'dve' or all_dve:
                            ops.append(lambda dst=dst, src=src, t=t:
                                       nc.vector.tensor_scalar_mul(
                                           dst, src, scal[t]))
                        else:
                            ops.append(lambda dst=dst, src=src, t=t:
                                       nc.scalar.mul(dst, src, scal[t]))
                    for g, _, _ in EW_FOLDS:
                        _, ctr, dst = aps(g, 0, f0, n)
                        ops.append(lambda dst=dst, ctr=ctr:
                                   nc.vector.tensor_add(dst, dst, ctr))
                    for si, (g, t, form) in enumerate(EW_MACS):
                        src, _, dst = aps(g, t, f0, n)
                        if form == 'stt' or (form == 'smul' and all_dve):
                            ops.append(lambda dst=dst, src=src, t=t:
                                       nc.vector.scalar_tensor_tensor(
                                           out=dst, in0=src, scalar=scal[t],
                                           in1=dst, op0=MULT, op1=ADD))
                        else:
                            tsl = tmp[0:PR, f0:f0 + n]
                            if form == 'smul' or use_act:
                                ops.append(lambda tsl=tsl, src=src, t=t:
                                           nc.scalar.mul(tsl, src, scal[t]))
                            else:
                                ops.append(lambda tsl=tsl, src=src, t=t:
                                           nc.vector.tensor_scalar_mul(
                                               tsl, src, scal[t]))
                            ops.append(lambda dst=dst, tsl=tsl:
                                       nc.vector.tensor_add(dst, dst, tsl))
                return ops

            def emit_pe(b, bg_ops=(), mid_hook=None, allpe_chunks=()):
                bg_ops = list(bg_ops)
                nbg = len(bg_ops)
                cen4, dir4 = cen4_t[b], dir4_t[b]
                y4 = y4_t.get(b)
                stag = None
                for chk in range(NCH):
                    r0 = chk * RPC
                    pt = ppool.tile([128, IPB * PCH], F32,
                                    name=f"ps_{b}_{chk}", tag="ps")

                    def mm_tap(wcol, kk, rhs, base, dy, dx, first=False):
                        # full-chunk matmul; shifted rhs reads pad zeros
                        # at the borders (SAME semantics), PSUM write is
                        # always contiguous [128, CHUNK].
                        wsl = wtb[:, wcol:wcol + 128]
                        rb = base + DOFF + (r0 + dy) * PW + dx
                        for i in range(IPB):
                            p0 = 32 * i
                            o = pt[:, i * PCH:i * PCH + CHUNK]
                            r = rhs[p0:p0 + kk, rb:rb + RPC * PW].rearrange(
                                "p (h w) -> p h w", w=PW)[:, :, 0:W]
                            nc.tensor.matmul(
                                o, wsl[p0:p0 + kk, :], r,
                                start=first, stop=False, tile_position=(p0, 0))

                    mm_tap(WB_CEN + 128 * 4, 32, cen4, 0, 0, 0, first=True)
                    for t in (0, 1, 2, 3, 5, 6, 7, 8):
                        mm_tap(WB_CEN + 128 * t, 32, cen4, 0,
                               t // 3 - 1, t % 3 - 1)
                    if chk in allpe_chunks:
                        # fully fused: every dir tap on the PE, no stage-2.
                        taps = [(g, t) for g in range(4) for t in range(5)]
                        for g, t in taps[:-1]:
                            dy, dx = dir_shift(g, t)
                            mm_tap(WB_DIR + 128 * (5 * g + t), 24, dir4,
                                   g * GS, dy, dx)
                        g, t = taps[-1]
                        dy, dx = dir_shift(g, t)
                        wcol = WB_DIR + 128 * (5 * g + t)
                        rb = g * GS + DOFF + (r0 + dy) * PW + dx
                        for i in range(IPB):
                            p0 = 32 * i
                            nc.tensor.matmul(
                                pt[:, i * PCH:i * PCH + CHUNK],
                                wtb[p0:p0 + 24, wcol:wcol + 128],
                                dir4[p0:p0 + 24, rb:rb + RPC * PW].rearrange(
                                    "p (h w) -> p h w", w=PW)[:, :, 0:W],
                                start=False, stop=True, tile_position=(p0, 0))
                    else:
                        for g, t in PE_TAPS:
                            dy, dx = dir_shift(g, t)
                            mm_tap(WB_DIR + 128 * (5 * g + t), 24, dir4,
                                   g * GS, dy, dx)
                        # stage-2 groups ordered to match y'' completion order
                        for gi, g in enumerate((1, 3, 0, 2)):
                            gb = g * GS + DOFF + r0 * PW
                            for i in range(IPB):
                                nc.tensor.matmul(
                                    pt[:, i * PCH:i * PCH + CHUNK],
                                    wtb[32 * i:32 * i + 24,
                                        WB_S2 + 128 * g:WB_S2 + 128 * (g + 1)],
                                    y4[32 * i:32 * i + 24,
                                       gb:gb + RPC * PW].rearrange(
                                           "p (h w) -> p h w", w=PW)[:, :, 0:W],
                                    start=False, stop=(gi == 3),
                                    tile_position=(32 * i, 0))

                    # evacuation: strided 4-image copy on ACT.  The last
                    # block stores per-chunk so the final store is small
                    # and starts as early as possible.
                    sg = 1 if b == nblk - 1 else SGRP
                    j = chk % sg
                    if j == 0:
                        stag = spool.tile([128, IPB * sg * CHUNK], BF16,
                                          name=f"st_{b}_{chk}", tag="st")
                    dstv = stag[:, :].rearrange(
                        "p (i f) -> p i f", i=IPB)[:, :, j * CHUNK:(j + 1) * CHUNK]
                    srcv = pt[:, :].rearrange("p (i f) -> p i f", i=IPB)[:, :, 0:CHUNK]
                    nc.scalar.copy(dstv, srcv)
                    if j == sg - 1:
                        lo = (b * NCH + chk - sg + 1) * IPB * CHUNK
                        nc.scalar.dma_start(
                            out=out[:, lo:lo + IPB * sg * CHUNK],
                            in_=stag[:, :])
                    # drain the next block's y''-prep ops front-loaded so
                    # its slice-1 is ready when block b+1's stage-2 starts.
                    frac = (30, 50, 70, 85, 100, 100)[chk]
                    take = nbg * frac // 100 - len([None] * 0) - (nbg - len(bg_ops))
                    for _ in range(max(0, take)):
                        bg_ops.pop(0)()
                    if chk == 2 and mid_hook is not None:
                        mid_hook()

            for b in range(nblk):
                emit_loads(b)
            # block 0 runs chunks 0-2 fully PE-fused, so its y''-prep
            # only needs rows 24-48 (consumed by chunks 3-5).
            for op in make_yprep(0, slices=((1200, 2400),)):
                op()
            for b in range(nblk):
                bg = (make_yprep(b + 1, all_dve=(b == 0))
                      if b + 1 < nblk else [])
                emit_pe(b, bg, mid_hook=None,
                        allpe_chunks=(0, 1, 2) if b == 0 else ())

    nc.compile()
    return nc


_NC_CACHE = {}


def _get_nc(nb):
    if nb not in _NC_CACHE:
        _NC_CACHE[nb] = build_nc(nb)
    return _NC_CACHE[nb]


def pack_inputs(x):
    """Pad/permute x into the device layouts (see module docstring)."""
    bf = mybir.dt.np(BF16)
    x = np.ascontiguousarray(x, np.float32).reshape(B, C, H, W).astype(bf)
    xcen = np.zeros((B // IPB, 128, CENF), bf)
    v = xcen[:, :, DOFF:DOFF + NROW].reshape(B // IPB, 128, H, PW)
    v[:, :, :, 0:W] = x[:, 0:32].reshape(B // IPB, 128, H, W)
    xdir = np.zeros((B // IPB, 128, DIRF), bf)
    rows = (np.arange(IPB)[:, None] * 32 + np.arange(DIR_IN)[None, :]).ravel()
    for g in range(4):
        v = xdir[:, :, g * GS + DOFF:g * GS + DOFF + NROW].reshape(
            B // IPB, 128, H, PW)
        v[:, rows, :, 0:W] = x[:, 32 + 24 * g:56 + 24 * g].reshape(
            B // IPB, IPB, DIR_IN, H, W).reshape(B // IPB, IPB * DIR_IN, H, W)
    return xcen, xdir


def unpack_output(outs):
    """outs: per-core [128, NBLK*NCH*IPB*CHUNK] f32 arrays; blocks 0-2
    are stored in (store-unit, image, SGRP*CHUNK) order, the last block
    per-chunk."""
    full = np.empty((B, C, H, W), np.float32)
    ns = NCH // SGRP
    pb = NCH * IPB * CHUNK
    for k, o in enumerate(outs):
        v = o[:, 0:(NBLK - 1) * pb].reshape(128, NBLK - 1, ns, IPB, SGRP * CHUNK)
        v = v.transpose(1, 3, 0, 2, 4).reshape(NB - IPB, C, H, W)
        full[k * NB:k * NB + NB - IPB] = v
        w = o[:, (NBLK - 1) * pb:].reshape(128, NCH, IPB, CHUNK)
        w = w.transpose(2, 0, 1, 3).reshape(IPB, C, H, W)
        full[k * NB + NB - IPB:(k + 1) * NB] = w
    return full


def kernel(x, cen_tensor, dir_tensor, cen2cen, par2cen, dia2cen, cen2dir, dir2dir,
           _trace=False):
    wtb_np, wts_np = build_weights(
        np.asarray(cen_tensor, np.float32), np.asarray(dir_tensor, np.float32),
        np.asarray(cen2cen, np.float32), np.asarray(par2cen, np.float32),
        np.asarray(dia2cen, np.float32), np.asarray(cen2dir, np.float32),
        np.asarray(dir2dir, np.float32))
    nc = _get_nc(NB)
    xcen, xdir = pack_inputs(np.asarray(x))
    nbb = NBLK
    in_maps = []
    for k in range(N_CORES):
        in_maps.append({
            "xcen": np.ascontiguousarray(xcen[k * nbb:(k + 1) * nbb]),
            "xdir": np.ascontiguousarray(xdir[k * nbb:(k + 1) * nbb]),
            "wtb": wtb_np, "wts": wts_np,
        })
    res = run_bass_kernel_spmd(nc, in_maps, list(range(N_CORES)), trace=_trace)
    outs = [np.asarray(res.results[k]["out"]).astype(np.float32)
            for k in range(N_CORES)]
    full = unpack_output(outs)
    if _trace:
        return full, res
    return full


# revision 80
# speedup vs baseline: 1.0396x; 1.0396x over previous
"""Trainium2 Bass kernel for DirectionalSeparableConv2D (v4).

Full-input contract: kernel(**inputs) takes the complete unsharded inputs
(x [128,128,48,48] plus the small weight tensors) and returns the full
[128,128,48,48] output. Internally shards batch 16-per-core across 8
NeuronCores (SPMD), with all weights replicated.

Design (vs the ~168-178us v3 baseline; measures ~116-120us):
  - Padded DRAM/SBUF x layout: rows are W+2=50 wide and image/group
    segments are separated by 100-col zero gaps, so every shifted tap
    (PE matmul rhs or vector op src) reads zeros at the SAME-conv
    borders.  No border clipping anywhere: every PSUM write is a full
    contiguous 384-col block and every elementwise tap is one flat
    dense 1D op (eligible for the DVE 2x/4x packed modes).
  - Engine split (LP-balanced): PE runs 20 packs per 8-row chunk
    (9 central taps, 7 fused dir taps, 4 stage-2 mixes) as 4-way
    tile_position-concurrent matmuls (~3.9 cols/cycle measured); DVE
    runs the remaining dir taps as tensor_scalar 4x muls + bf16
    tensor_tensor 2x adds + two 1x STT MACs; ACT does the PSUM
    evacuation and two tap muls.  GpSimd is left idle ON PURPOSE: its
    SBUF port is shared with DVE and concurrent GpSimd tensor ops
    halve DVE throughput (measured ~2.3x slowdown on overlapped ops).
  - Center-tap folding: y''_g = x + sum_t (k_t/k2) shift_t(x); the
    stage-2 weight columns are pre-scaled by k2[c] on the host, so the
    center taps cost nothing on-device.
  - DMA: whole-block loads ([128, 5.2KB] cen / [128, 20.2KB] dir
    lines, all 16 SDMA engines engaged) and device-native stores
    ([128, 6KB] lines) that the host unpermutes; ~25 GB/s per
    descriptor stream vs ~14 with the v3 layout (2.2KB packets).
  - Pipeline fill: block 0's chunks 0-2 run fully PE-fused (all 20 dir
    taps as matmuls, no stage-2) so nothing gates on the serial DVE
    y''-prep chain while the pipe fills; its y''-prep covers only rows
    24-48.  ACT FIFO is strict, so cross-block ACT ops that wait on a
    future DMA must not precede this block's evacuations; yprep for
    block 1 runs all-DVE for that reason.
"""

import numpy as np

import concourse.bacc as bacc
import concourse.mybir as mybir
import concourse.tile as tile
from concourse.bass_utils import run_bass_kernel_spmd

F32 = mybir.dt.float32
BF16 = mybir.dt.bfloat16

# Problem shapes (hardcoded per contract).
B, C, H, W = 128, 128, 48, 48
HW = H * W
CEN_IN, DIR_IN = 32, 24
N_CORES = 8

# Per-core tiling.
NB = B // N_CORES          # images per core (16)
IPB = 4                    # images per block (one per 32-partition slot)
NBLK = NB // IPB           # 4 blocks
RPC = 8                    # rows per PSUM chunk
CHUNK = RPC * W            # 384
PCH = 512                  # PSUM per-image slice stride (bank aligned)
NCH = H // RPC             # 6 chunks per image
SGRP = 2                   # chunks buffered per output store
PR = 96 + DIR_IN           # partition extent covering all dir slots (120)

# Padded plane geometry: rows are PW=50 wide (2 pad cols absorb dx in
# [-2,2] reads); segments start with 100 zero cols (absorb dy in [-2,2]
# row reads); DOFF is the offset of pixel (0,0) within a segment.
PW = W + 2                 # 50
GS = 100 + 2 + H * PW      # 2502 segment stride
DOFF = 102                 # data base within a segment
CENF = GS + 100            # cen tile free size (1 segment + tail zeros)
DIRF = 4 * GS + 104        # dir tile free size (4 group segments + tail)
NROW = H * PW              # 2400 flat cols per plane (incl in-row pads)

# Dir-group tap geometry: group g shift for tap t (t=0..4, center t=2).
#   g=0 horizontal (0, t-2); g=1 vertical (t-2, 0);
#   g=2 diagonal (t-2, t-2); g=3 anti-diagonal (t-2, 2-t).
def dir_shift(g, t):
    d = t - 2
    return [(0, d), (d, 0), (d, d), (d, -d)][g]

# PE keeps 8 dir taps fused with the mix (LP-balanced against DVE/ACT);
# the remaining 8 taps (plus the 4 folded centers) run elementwise:
#   fold-dve:   y = k~t*shift(x)  (tensor_scalar mul 4x; even offsets)
#   fold-act:   y = k~t*shift(x)  (ACT mul; handles odd offsets)
#   then        y += x            (DVE tensor_tensor add, 2x)
#   mac-dve:    tmp = k~t*shift(x) (4x); y += tmp (2x)
#   mac-stt:    y = k~t*shift(x) + y  (DVE STT 1x; odd offsets)
# GpSimd stays idle: its SBUF port is shared with DVE and concurrent
# GpSimd tensor ops halve DVE throughput (measured).
PE_TAPS = [(0, 0), (0, 1), (0, 3), (1, 0), (1, 1), (1, 3), (2, 0)]
# folds: (g, t, engine); first elementwise tap per group, absorbs y=..
EW_FOLDS = [(1, 4, 'dve'), (3, 0, 'dve'), (0, 4, 'act'), (2, 3, 'act')]
# macs: (g, t, form); tmul = DVE ts-mul + add, smul = ACT mul + DVE add
EW_MACS = [(3, 4, 'tmul'), (3, 1, 'stt'), (3, 3, 'stt'),
           (2, 4, 'tmul'), (2, 1, 'stt')]
SCAL_TAPS = [0, 1, 3, 4]   # scalar columns (k_t/k2) used by elementwise taps

# bf16 weight bundle layout (columns).  The fused-dir section holds ALL
# 20 (g, t) taps: blocks 1-3 index the PE_TAPS subset; block 0 runs
# fully fused on the PE (no y''/stage-2) to avoid gating its chunks on
# the serial DVE y''-prep chain during pipeline fill.
WB_CEN = 0                 # 9 central taps x 128 cols
WB_DIR = 9 * 128           # 20 fused dir taps x 128 cols
WB_S2 = WB_DIR + 20 * 128  # 4 stage-2 blocks x 128 cols
NWTB = WB_S2 + 4 * 128


def build_mix(cen2cen, par2cen, dia2cen, cen2dir, dir2dir):
    mix = np.zeros((C, C), np.float32)
    mix[0:32, 0:32] = cen2cen
    mix[0:32, 32:56] = par2cen
    mix[0:32, 56:80] = par2cen
    mix[0:32, 80:104] = dia2cen
    mix[0:32, 104:128] = dia2cen
    for g in range(4):
        r = 32 + 24 * g
        mix[r:r + 24, 0:32] = cen2dir
        mix[r:r + 24, r:r + 24] = dir2dir
    return mix


def build_weights(cen_tensor, dir_tensor, cen2cen, par2cen, dia2cen, cen2dir, dir2dir):
    mix = build_mix(cen2cen, par2cen, dia2cen, cen2dir, dir2dir)
    bf = mybir.dt.np(BF16)
    # Clamp the center tap away from 0 so the fold ratios stay finite;
    # using the clamped value consistently in both the ratios and the
    # stage-2 scale keeps the math exact up to the 1e-8 perturbation.
    k2 = dir_tensor[:, 2]
    k2c = np.where(np.abs(k2) < 1e-8, np.where(k2 < 0, -1e-8, 1e-8), k2)
    wtb = np.zeros((128, NWTB), bf)
    for t in range(9):
        blk = (mix[:, 0:32] * cen_tensor[:, t // 3, t % 3][None, :]).T.astype(bf)
        for i in range(IPB):
            wtb[32 * i:32 * i + 32, WB_CEN + 128 * t:WB_CEN + 128 * (t + 1)] = blk
    for g in range(4):
        for t in range(5):
            j = 5 * g + t
            cols = slice(32 + 24 * g, 56 + 24 * g)
            blk = (mix[:, cols] * dir_tensor[:, t][None, :]).T.astype(bf)
            for i in range(IPB):
                wtb[32 * i:32 * i + 24,
                    WB_DIR + 128 * j:WB_DIR + 128 * (j + 1)] = blk
    for g in range(4):
        s2 = (mix[:, 32 + 24 * g:56 + 24 * g] * k2c[None, :]).T.astype(bf)
        for i in range(IPB):
            wtb[32 * i:32 * i + 24, WB_S2 + 128 * g:WB_S2 + 128 * (g + 1)] = s2
    # f32 per-partition scalar columns: k_t/k2 for the elementwise taps.
    wts = np.zeros((128, len(SCAL_TAPS)), np.float32)
    for j, t in enumerate(SCAL_TAPS):
        for i in range(IPB):
            wts[32 * i:32 * i + 24, j] = dir_tensor[:, t] / k2c
    return wtb, wts


def build_nc(nb=NB):
    """Emit the per-core Bass program for nb images."""
    assert nb % IPB == 0
    nblk = nb // IPB
    nc = bacc.Bacc("TRN2", target_bir_lowering=False, debug=False)

    xcen = nc.dram_tensor("xcen", [nblk, 128, CENF], BF16, kind="ExternalInput")
    xdir = nc.dram_tensor("xdir", [nblk, 128, DIRF], BF16, kind="ExternalInput")
    wtbd = nc.dram_tensor("wtb", [128, NWTB], BF16, kind="ExternalInput")
    wtsd = nc.dram_tensor("wts", [128, len(SCAL_TAPS)], F32, kind="ExternalInput")
    # Device-native output: staging tiles dumped verbatim; host unpermutes.
    out = nc.dram_tensor("out", [128, nblk * (NCH // SGRP) * IPB * SGRP * CHUNK],
                         BF16, kind="ExternalOutput")

    MULT = mybir.AluOpType.mult
    ADD = mybir.AluOpType.add

    with tile.TileContext(nc) as tc:
        with (
            tc.tile_pool(name="wpool", bufs=1) as wpool,
            tc.tile_pool(name="cpool", bufs=4) as cpool,
            tc.tile_pool(name="dpool", bufs=4) as dpool,
            tc.tile_pool(name="ypool", bufs=2) as ypool,
            tc.tile_pool(name="tpool", bufs=2) as tpool,
            tc.tile_pool(name="spool", bufs=2) as spool,
            tc.tile_pool(name="ppool", bufs=2, space="PSUM") as ppool,
        ):
            wtb = wpool.tile([128, NWTB], BF16)
            # central-tap weight columns first: they gate the first matmuls
            nc.scalar.dma_start(out=wtb[:, 0:WB_DIR], in_=wtbd[:, 0:WB_DIR])
            nc.scalar.dma_start(out=wtb[:, WB_DIR:NWTB], in_=wtbd[:, WB_DIR:NWTB])
            wts = wpool.tile([128, len(SCAL_TAPS)], F32)
            nc.scalar.dma_start(out=wts[:, :], in_=wtsd[:, :])
            scal = {t: wts[0:PR, j:j + 1] for j, t in enumerate(SCAL_TAPS)}

            cen4_t, dir4_t, y4_t = {}, {}, {}

            def emit_loads(b):
                cen4 = cpool.tile([128, CENF], BF16, name=f"cen4_{b}", tag="cen4")
                dir4 = dpool.tile([128, DIRF], BF16, name=f"dir4_{b}", tag="dir4")
                cen4_t[b], dir4_t[b] = cen4, dir4
                if b == 0:
                    # chunk 0 reads cen rows <= 9 only: land those first
                    cs = DOFF + 10 * PW
                    nc.sync.dma_start(out=cen4[:, 0:cs], in_=xcen[b, :, 0:cs])
                    nc.sync.dma_start(out=cen4[:, cs:CENF],
                                      in_=xcen[b, :, cs:CENF])
                else:
                    nc.sync.dma_start(out=cen4[:, :], in_=xcen[b, :, :])
                if b == 0:
                    for g in range(4):
                        hi = (g + 1) * GS if g < 3 else DIRF
                        nc.sync.dma_start(out=dir4[:, g * GS:hi],
                                          in_=xdir[b, :, g * GS:hi])
                else:
                    nc.sync.dma_start(out=dir4[:, :], in_=xdir[b, :, :])

            def make_yprep(b, slices=((0, 800), (800, 2400)), use_act=False,
                           all_dve=False):
                """Build the y''-prep op list (emission deferred)."""
                dir4 = dir4_t[b]
                y4 = ypool.tile([128, DIRF], BF16, name=f"y4_{b}", tag="y4")
                y4_t[b] = y4
                tmp = tpool.tile([128, NROW], BF16, name=f"tmp_{b}", tag="tmp")
                ops = []

                def aps(g, t, f0, n):
                    dy, dx = dir_shift(g, t)
                    base = g * GS + DOFF + f0
                    off = dy * PW + dx
                    return (dir4[0:PR, base + off:base + off + n],
                            dir4[0:PR, base:base + n],
                            y4[0:PR, base:base + n])

                for f0, f1 in slices:
                    n = f1 - f0
                    for g, t, eng in EW_FOLDS:
                        src, _, dst = aps(g, t, f0, n)
                        if eng == 'dve' or all_dve:
                            ops.append(lambda dst=dst, src=src, t=t:
                                       nc.vector.tensor_scalar_mul(
                                           dst, src, scal[t]))
                        else:
                            ops.append(lambda dst=dst, src=src, t=t:
                                       nc.scalar.mul(dst, src, scal[t]))
                    for g, _, _ in EW_FOLDS:
                        _, ctr, dst = aps(g, 0, f0, n)
                        ops.append(lambda dst=dst, ctr=ctr:
                                   nc.vector.tensor_add(dst, dst, ctr))
                    for si, (g, t, form) in enumerate(EW_MACS):
                        src, _, dst = aps(g, t, f0, n)
                        if form == 'stt' or (form == 'smul' and all_dve):
                            ops.append(lambda dst=dst, src=src, t=t:
                                       nc.vector.scalar_tensor_tensor(
                                           out=dst, in0=src, scalar=scal[t],
                                           in1=dst, op0=MULT, op1=ADD))
                        else:
                            tsl = tmp[0:PR, f0:f0 + n]
                            if form == 'smul' or use_act:
                                ops.append(lambda tsl=tsl, src=src, t=t:
                                           nc.scalar.mul(tsl, src, scal[t]))
                            else:
                                ops.append(lambda tsl=tsl, src=src, t=t:
                                           nc.vector.tensor_scalar_mul(
                                               tsl, src, scal[t]))
                            ops.append(lambda dst=dst, tsl=tsl:
                                       nc.vector.tensor_add(dst, dst, tsl))
                return ops

            def emit_pe(b, bg_ops=(), mid_hook=None, allpe_chunks=()):
                bg_ops = list(bg_ops)
                nbg = len(bg_ops)
                cen4, dir4 = cen4_t[b], dir4_t[b]
                y4 = y4_t.get(b)
                stag = None
                for chk in range(NCH):
                    r0 = chk * RPC
                    pt = ppool.tile([128, IPB * PCH], F32,
                                    name=f"ps_{b}_{chk}", tag="ps")

                    def mm_tap(wcol, kk, rhs, base, dy, dx, first=False):
                        # full-chunk matmul; shifted rhs reads pad zeros
                        # at the borders (SAME semantics), PSUM write is
                        # always contiguous [128, CHUNK].
                        wsl = wtb[:, wcol:wcol + 128]
                        rb = base + DOFF + (r0 + dy) * PW + dx
                        for i in range(IPB):
                            p0 = 32 * i
                            o = pt[:, i * PCH:i * PCH + CHUNK]
                            r = rhs[p0:p0 + kk, rb:rb + RPC * PW].rearrange(
                                "p (h w) -> p h w", w=PW)[:, :, 0:W]
                            nc.tensor.matmul(
                                o, wsl[p0:p0 + kk, :], r,
                                start=first, stop=False, tile_position=(p0, 0))

                    mm_tap(WB_CEN + 128 * 4, 32, cen4, 0, 0, 0, first=True)
                    for t in (0, 1, 2, 3, 5, 6, 7, 8):
                        mm_tap(WB_CEN + 128 * t, 32, cen4, 0,
                               t // 3 - 1, t % 3 - 1)
                    if chk in allpe_chunks:
                        # fully fused: every dir tap on the PE, no stage-2.
                        taps = [(g, t) for g in range(4) for t in range(5)]
                        for g, t in taps[:-1]:
                            dy, dx = dir_shift(g, t)
                            mm_tap(WB_DIR + 128 * (5 * g + t), 24, dir4,
                                   g * GS, dy, dx)
                        g, t = taps[-1]
                        dy, dx = dir_shift(g, t)
                        wcol = WB_DIR + 128 * (5 * g + t)
                        rb = g * GS + DOFF + (r0 + dy) * PW + dx
                        for i in range(IPB):
                            p0 = 32 * i
                            nc.tensor.matmul(
                                pt[:, i * PCH:i * PCH + CHUNK],
                                wtb[p0:p0 + 24, wcol:wcol + 128],
                                dir4[p0:p0 + 24, rb:rb + RPC * PW].rearrange(
                                    "p (h w) -> p h w", w=PW)[:, :, 0:W],
                                start=False, stop=True, tile_position=(p0, 0))
                    else:
                        for g, t in PE_TAPS:
                            dy, dx = dir_shift(g, t)
                            mm_tap(WB_DIR + 128 * (5 * g + t), 24, dir4,
                                   g * GS, dy, dx)
                        # stage-2 groups ordered to match y'' completion order
                        for gi, g in enumerate((1, 3, 0, 2)):
                            gb = g * GS + DOFF + r0 * PW
                            for i in range(IPB):
                                nc.tensor.matmul(
                                    pt[:, i * PCH:i * PCH + CHUNK],
                                    wtb[32 * i:32 * i + 24,
                                        WB_S2 + 128 * g:WB_S2 + 128 * (g + 1)],
                                    y4[32 * i:32 * i + 24,
                                       gb:gb + RPC * PW].rearrange(
                                           "p (h w) -> p h w", w=PW)[:, :, 0:W],
                                    start=False, stop=(gi == 3),
                                    tile_position=(32 * i, 0))

                    # evacuation: strided 4-image copy on ACT.  The last
                    # block stores per-chunk so the final store is small
                    # and starts as early as possible.
                    sg = 1 if b == nblk - 1 else SGRP
                    j = chk % sg
                    if j == 0:
                        stag = spool.tile([128, IPB * sg * CHUNK], BF16,
                                          name=f"st_{b}_{chk}", tag="st")
                    dstv = stag[:, :].rearrange(
                        "p (i f) -> p i f", i=IPB)[:, :, j * CHUNK:(j + 1) * CHUNK]
                    srcv = pt[:, :].rearrange("p (i f) -> p i f", i=IPB)[:, :, 0:CHUNK]
                    nc.scalar.copy(dstv, srcv)
                    if j == sg - 1:
                        lo = (b * NCH + chk - sg + 1) * IPB * CHUNK
                        nc.scalar.dma_start(
                            out=out[:, lo:lo + IPB * sg * CHUNK],
                            in_=stag[:, :])
                    # drain the next block's y''-prep ops front-loaded so
                    # its slice-1 is ready when block b+1's stage-2 starts.
                    frac = (30, 50, 70, 85, 100, 100)[chk]
                    take = nbg * frac // 100 - len([None] * 0) - (nbg - len(bg_ops))
                    for _ in range(max(0, take)):
                        bg_ops.pop(0)()
                    if chk == 2 and mid_hook is not None:
                        mid_hook()

            for b in range(nblk):
                emit_loads(b)
            # block 0 runs chunks 0-1 fully PE-fused (all-PE chunks cost
            # ~6us each from LDWEIGHTS pressure), so its y''-prep covers
            # rows 16-48 (consumed by chunks 2-5).
            for op in make_yprep(0, slices=((800, 1600), (1600, 2400))):
                op()
            for b in range(nblk):
                bg = (make_yprep(b + 1, all_dve=(b == 0))
                      if b + 1 < nblk else [])
                emit_pe(b, bg, mid_hook=None,
                        allpe_chunks=(0, 1) if b == 0 else ())

    nc.compile()
    return nc


_NC_CACHE = {}


def _get_nc(nb):
    if nb not in _NC_CACHE:
        _NC_CACHE[nb] = build_nc(nb)
    return _NC_CACHE[nb]


def pack_inputs(x):
    """Pad/permute x into the device layouts (see module docstring)."""
    bf = mybir.dt.np(BF16)
    x = np.ascontiguousarray(x, np.float32).reshape(B, C, H, W).astype(bf)
    xcen = np.zeros((B // IPB, 128, CENF), bf)
    v = xcen[:, :, DOFF:DOFF + NROW].reshape(B // IPB, 128, H, PW)
    v[:, :, :, 0:W] = x[:, 0:32].reshape(B // IPB, 128, H, W)
    xdir = np.zeros((B // IPB, 128, DIRF), bf)
    rows = (np.arange(IPB)[:, None] * 32 + np.arange(DIR_IN)[None, :]).ravel()
    for g in range(4):
        v = xdir[:, :, g * GS + DOFF:g * GS + DOFF + NROW].reshape(
            B // IPB, 128, H, PW)
        v[:, rows, :, 0:W] = x[:, 32 + 24 * g:56 + 24 * g].reshape(
            B // IPB, IPB, DIR_IN, H, W).reshape(B // IPB, IPB * DIR_IN, H, W)
    return xcen, xdir


def unpack_output(outs):
    """outs: per-core [128, NBLK*NCH*IPB*CHUNK] f32 arrays; blocks 0-2
    are stored in (store-unit, image, SGRP*CHUNK) order, the last block
    per-chunk."""
    full = np.empty((B, C, H, W), np.float32)
    ns = NCH // SGRP
    pb = NCH * IPB * CHUNK
    for k, o in enumerate(outs):
        v = o[:, 0:(NBLK - 1) * pb].reshape(128, NBLK - 1, ns, IPB, SGRP * CHUNK)
        v = v.transpose(1, 3, 0, 2, 4).reshape(NB - IPB, C, H, W)
        full[k * NB:k * NB + NB - IPB] = v
        w = o[:, (NBLK - 1) * pb:].reshape(128, NCH, IPB, CHUNK)
        w = w.transpose(2, 0, 1, 3).reshape(IPB, C, H, W)
        full[k * NB + NB - IPB:(k + 1) * NB] = w
    return full


def kernel(x, cen_tensor, dir_tensor, cen2cen, par2cen, dia2cen, cen2dir, dir2dir,
           _trace=False):
    wtb_np, wts_np = build_weights(
        np.asarray(cen_tensor, np.float32), np.asarray(dir_tensor, np.float32),
        np.asarray(cen2cen, np.float32), np.asarray(par2cen, np.float32),
        np.asarray(dia2cen, np.float32), np.asarray(cen2dir, np.float32),
        np.asarray(dir2dir, np.float32))
    nc = _get_nc(NB)
    xcen, xdir = pack_inputs(np.asarray(x))
    nbb = NBLK
    in_maps = []
    for k in range(N_CORES):
        in_maps.append({
            "xcen": np.ascontiguousarray(xcen[k * nbb:(k + 1) * nbb]),
            "xdir": np.ascontiguousarray(xdir[k * nbb:(k + 1) * nbb]),
            "wtb": wtb_np, "wts": wts_np,
        })
    res = run_bass_kernel_spmd(nc, in_maps, list(range(N_CORES)), trace=_trace)
    outs = [np.asarray(res.results[k]["out"]).astype(np.float32)
            for k in range(N_CORES)]
    full = unpack_output(outs)
    if _trace:
        return full, res
    return full
